# revision 21
# baseline (speedup 1.0000x reference)
"""ANI-style per-species MLP (MoE routing) on 8 Trainium2 NeuronCores.

Strategy:
  - Data-parallel over conformations: 256 conformations per core.
  - Routing done host-side as part of sharding: within each
    (core, 64-conformation block), atoms are stably grouped by species and
    padded to a fixed slot count PB. The device then runs dense per-species
    MLPs over contiguous slot segments (1x compute instead of 4x).
  - Per-conformation sums are computed on device with a one-hot
    slot->conformation matmul (pad slots have all-zero rows, so their
    garbage outputs never reach the energies).
  - CELU(x, a) = relu(x+c) + min(a*exp((x+c)/a), a) - a:
    one ScalarE Exp (bias/scale folded into the activation instruction),
    one VectorE tensor_scalar min, one fused VectorE scalar_tensor_tensor
    (max-with--c then add). Constant offsets (c - a) are folded into the
    next layer's bias on the host.
"""

import sys

if "/opt/trn_rl_repo" not in sys.path:
    sys.path.insert(0, "/opt/trn_rl_repo")

import numpy as np
import ml_dtypes

# ---- problem constants (hardcoded per spec) --------------------------------
S, D, H1, H2, H3 = 4, 384, 160, 128, 96
B, A = 2048, 64
ALPHA = 0.1
LN_ALPHA = float(np.log(ALPHA))
INV_ALPHA = 1.0 / ALPHA

NCORES = 8
BC = B // NCORES           # conformations per core (256)
NBLK = 4                   # conformation blocks per core
CBLK = BC // NBLK          # conformations per block (64)
APB = CBLK * A             # atoms per block (4096)
SG = NBLK * S              # segments per core (16), sg = k*S + s
TILE = 384                 # slots per compute tile
PB0 = 1152                 # default padded slots per (species, block) segment

BF16 = ml_dtypes.bfloat16

_BUILD_CACHE = {}


def _register_celu_tail():
    """Register a fused custom DVE op: out = max(in0, s0) + min(in1, s1).

    This computes the whole CELU tail (relu part + clamped exp part) in a
    single VectorE instruction; s0 is the per-partition -c bias AP.
    """
    from concourse import dve_ops
    from concourse.dve_spec import Spec, Src0, Src1, C0, C1, maxx, minn, lower
    from concourse.dve_uop import DveOpSpec

    for op in dve_ops.OPS:
        if op.name == "CELU_TAIL_ANT":
            return op
    spec = Spec(
        body=maxx(Src0, C0) + minn(Src1, C1),
        reference=lambda in0, in1, s0, s1, imm2:
            np.maximum(in0, s0) + np.minimum(in1, s1),
    )
    row = dve_ops._CUSTOM_DVE_ROW_BASE + len(dve_ops.OPS)
    shas = {}
    for ver in ("v3", "v4"):
        tmp = DveOpSpec(name="CELU_TAIL_ANT", opcode=row,
                        uops=lower(spec, ver=ver), rd1_en=True)
        shas[ver] = tmp.sha(ver)
    op = dve_ops.DveOp("CELU_TAIL_ANT", spec, subdim=False, uops_sha=shas)
    dve_ops.OPS.append(op)
    dve_ops._SUB_OPCODE_FOR_NAME[op.name] = row
    dve_ops.CUSTOM_DVE_SPECS[op.name] = spec
    return op


def _build(PB):
    """Build + compile the per-core Bass graph for padded segment size PB."""
    if PB in _BUILD_CACHE:
        return _BUILD_CACHE[PB]

    import concourse.bass as bass
    import concourse.bacc as bacc
    import concourse.mybir as mybir
    import concourse.tile as tile
    from contextlib import ExitStack

    dt = mybir.dt
    F32, BF = dt.float32, dt.bfloat16
    AF = mybir.ActivationFunctionType
    OP = mybir.AluOpType

    NT = PB // TILE            # tiles per segment
    NJ = PB // 128             # 128-slot chunks per segment
    JPT = TILE // 128          # 128-slot chunks per tile
    H1B = H1 - 128             # second-chunk width of layer-1 output (32)

    nc = bacc.Bacc("TRN2", target_bir_lowering=False, debug=False,
                   num_devices=NCORES)

    xs = nc.declare_dram_parameter("xs", [SG, D, PB], BF, isOutput=False)
    cm = nc.declare_dram_parameter("cm", [SG, 128, NJ * CBLK], BF,
                                   isOutput=False)
    w1 = nc.declare_dram_parameter("w1", [S, D, H1], BF, isOutput=False)
    w2 = nc.declare_dram_parameter("w2", [S, H1, H2], BF, isOutput=False)
    w2b3 = nc.declare_dram_parameter("w2b3", [S, 3 * (H1 - 128), H2], BF,
                                     isOutput=False)
    w3 = nc.declare_dram_parameter("w3", [S, H2, H3], BF, isOutput=False)
    w4 = nc.declare_dram_parameter("w4", [S, H3, 1], BF, isOutput=False)
    # consolidated pre-folded biases (partition-major, one column per kind/s)
    bias_a = nc.declare_dram_parameter("bias_a", [128, 5 * S], F32,
                                       isOutput=False)
    bias_b = nc.declare_dram_parameter("bias_b", [3 * H1B, 4 * S], F32,
                                       isOutput=False)
    out = nc.declare_dram_parameter("out", [NBLK, CBLK], F32, isOutput=True)

    with tile.TileContext(nc) as tc:
        with ExitStack() as ctx:
            wpool = ctx.enter_context(tc.tile_pool(name="wpool", bufs=1))
            xpool = ctx.enter_context(tc.tile_pool(name="xpool", bufs=3))
            cpool = ctx.enter_context(tc.tile_pool(name="cpool", bufs=3))
            hpool = ctx.enter_context(tc.tile_pool(name="hpool", bufs=4))
            opool = ctx.enter_context(tc.tile_pool(name="opool", bufs=2))
            pp = ctx.enter_context(
                tc.tile_pool(name="pp", bufs=1, space=bass.MemorySpace.PSUM))
            pp2 = ctx.enter_context(
                tc.tile_pool(name="pp2", bufs=2, space=bass.MemorySpace.PSUM))

            # ---- load weights + biases once (persistent tiles) ----
            def wtile(shape, dtp, tag, src):
                t_ = wpool.tile(shape, dtp, tag=tag, name=tag)
                nc.sync.dma_start(t_[:], src)
                return t_

            w1t, w2at, w2bt, w3t, w4t = {}, {}, {}, {}, {}
            b1ea, b1eb, b1nat, b1nb = {}, {}, {}, {}
            b2et, b2nt, b3et, b3nt, c4tt = {}, {}, {}, {}, {}

            def load_weights(s):
                for kc in range(3):
                    w1t[s, kc] = wtile([128, H1], BF, f"w1_{s}_{kc}",
                                       w1[s, kc * 128:(kc + 1) * 128, :])
                w2at[s] = wtile([128, H2], BF, f"w2a_{s}", w2[s, 0:128, :])
                w2bt[s] = wtile([3 * H1B, H2], BF, f"w2b_{s}",
                                w2b3[s, :, :])
                w3t[s] = wtile([128, H3], BF, f"w3_{s}", w3[s, :, :])
                w4t[s] = wtile([H3, 1], BF, f"w4_{s}", w4[s, :, :])
                b1ea[s] = wtile([128, 1], F32, f"b1ea_{s}",
                                bias_a[:, 0 * S + s:0 * S + s + 1])
                b1nat[s] = wtile([128, 1], F32, f"b1na_{s}",
                                 bias_a[:, 1 * S + s:1 * S + s + 1])
                b2et[s] = wtile([128, 1], F32, f"b2e_{s}",
                                bias_a[:, 2 * S + s:2 * S + s + 1])
                b2nt[s] = wtile([128, 1], F32, f"b2n_{s}",
                                bias_a[:, 3 * S + s:3 * S + s + 1])
                c4tt[s] = wtile([128, 1], F32, f"c4_{s}",
                                bias_a[:, 4 * S + s:4 * S + s + 1])
                b1eb[s] = wtile([3 * H1B, 1], F32, f"b1eb_{s}",
                                bias_b[:, 0 * S + s:0 * S + s + 1])
                b1nb[s] = wtile([3 * H1B, 1], F32, f"b1nb_{s}",
                                bias_b[:, 1 * S + s:1 * S + s + 1])
                b3et[s] = wtile([H3, 1], F32, f"b3e_{s}",
                                bias_b[0:H3, 2 * S + s:2 * S + s + 1])
                b3nt[s] = wtile([H3, 1], F32, f"b3n_{s}",
                                bias_b[0:H3, 3 * S + s:3 * S + s + 1])

            celu_tail = _register_celu_tail()

            def celu(z, be_ap, bn_ap, mode, tagp, P, FD):
                """h' = max(z, -c) + min(a*exp((z+c)/a), a); model offset
                c - alpha is folded into the next layer's bias (bn_ap=-c)."""
                E = hpool.tile([P, FD], BF, tag=f"{tagp}_E", name=f"{tagp}_E")
                nc.scalar.activation(E[:], z[:], AF.Exp,
                                     bias=be_ap, scale=INV_ALPHA)
                h = hpool.tile([P, FD], BF, tag=f"{tagp}_h", name=f"{tagp}_h")
                nc.vector._custom_dve(celu_tail, out=h[:], in0=z[:],
                                      in1=E[:], s0=bn_ap, s1=ALPHA)
                return h

            # ---- main loop ----
            for k in range(NBLK):
                pe = pp.tile([1, CBLK], F32, tag="pe", name="pe")
                emm = 0
                for s in range(S):
                    sg = k * S + s
                    if k == 0:
                        load_weights(s)
                    ct = cpool.tile([128, NJ * CBLK], BF, tag="ct", name="ct")
                    nc.sync.dma_start(ct[:], cm[sg, :, :])
                    # stage the whole segment's X (feature-major), one DMA
                    # per 128-feature chunk, triggered on the gpsimd queue
                    xseg = []
                    for kc in range(3):
                        xt = xpool.tile([128, PB], BF, tag=f"x{kc}",
                                        name=f"x{kc}")
                        nc.gpsimd.dma_start(
                            xt[:], xs[sg, kc * 128:(kc + 1) * 128, :])
                        xseg.append(xt)
                    # layer-1 second chunk (32 rows) merged across the
                    # segment's tiles via column tiling -> one celu
                    p1b = pp.tile([32 * NT, TILE], F32, tag="p1b",
                                  name="p1b")
                    for kc in range(3):
                        for t in range(NT):
                            nc.tensor.matmul(
                                p1b[32 * t:32 * (t + 1), :],
                                w1t[s, kc][:, 128:H1],
                                xseg[kc][:, t * TILE:(t + 1) * TILE],
                                start=(kc == 0), stop=(kc == 2),
                                tile_position=(0, 32 * t))
                    h1b = celu(p1b, b1eb[s][:], b1nb[s][:],
                               "h1b", "dve", 32 * NT, TILE)
                    for t in range(NT):
                        tsl = slice(t * TILE, (t + 1) * TILE)
                        p1a = pp2.tile([128, TILE], F32, tag="p1a",
                                       name="p1a")
                        for kc in range(3):
                            nc.tensor.matmul(p1a[:], w1t[s, kc][:, 0:128],
                                             xseg[kc][:, tsl],
                                             start=(kc == 0), stop=(kc == 2))
                        h1a = celu(p1a, b1ea[s][:], b1nat[s][:], "dve",
                                   "h1a", 128, TILE)
                        p2 = pp2.tile([128, TILE], F32, tag="p2", name="p2")
                        nc.tensor.matmul(p2[:], w2at[s][:], h1a[:],
                                         start=True, stop=False)
                        nc.tensor.matmul(p2[:],
                                         w2bt[s][32 * t:32 * (t + 1), :],
                                         h1b[32 * t:32 * (t + 1), :],
                                         start=False, stop=True)
                        h2 = celu(p2, b2et[s][:], b2nt[s][:], "dve",
                                  "h2", 128, TILE)
                        p3 = pp.tile([H3, TILE], F32, tag="p3", name="p3")
                        nc.tensor.matmul(p3[:], w3t[s][:], h2[:],
                                         start=True, stop=True)
                        h3 = celu(p3, b3et[s][:], b3nt[s][:], "dve",
                                  "h3", H3, TILE)
                        py = pp.tile([128, JPT], F32, tag="py", name="py")
                        for c in range(JPT):
                            nc.tensor.matmul(py[:, c:c + 1],
                                             h3[:, c * 128:(c + 1) * 128],
                                             w4t[s][:],
                                             start=True, stop=True)
                        ysb = hpool.tile([128, JPT], BF, tag="ysb",
                                         name="ysb")
                        nc.scalar.activation(ysb[:], py[:], AF.Identity,
                                             bias=c4tt[s])
                        for c in range(JPT):
                            j = t * JPT + c
                            nc.tensor.matmul(
                                pe[:], ysb[:, c:c + 1],
                                ct[:, j * CBLK:(j + 1) * CBLK],
                                start=(emm == 0), stop=(emm == S * NJ - 1))
                            emm += 1
                esb = opool.tile([1, CBLK], F32, tag="esb", name="esb")
                nc.vector.tensor_copy(esb[:], pe[:])
                nc.sync.dma_start(out[k:k + 1, :], esb[:])

    nc.compile()
    _BUILD_CACHE[PB] = nc
    return nc


def _prep_inputs(species, aev, W1, b1, W2, b2, W3, b3, W4, b4):
    """Host-side routing/sharding + bias folding. Returns (PB, in_maps)."""
    species = np.asarray(species)
    aev = np.asarray(aev, dtype=np.float32)

    sp = species.reshape(NCORES, NBLK, APB)
    av = aev.reshape(NCORES, NBLK, APB, D)

    # padded segment size (multiple of TILE), robust to species-count skew
    counts = np.zeros((NCORES, NBLK, S), dtype=np.int64)
    for s in range(S):
        counts[:, :, s] = (sp == s).sum(axis=2)
    maxc = int(counts.max())
    PB = max(PB0, -(-maxc // TILE) * TILE)
    NJ = PB // 128

    xs = np.zeros((NCORES, SG, D, PB), dtype=BF16)
    cmh = np.zeros((NCORES, SG, 128, NJ * CBLK), dtype=BF16)
    for c in range(NCORES):
        for k in range(NBLK):
            spk = sp[c, k]
            avk = av[c, k]
            for s in range(S):
                idx = np.flatnonzero(spk == s)
                n = idx.size
                sg = k * S + s
                xs[c, sg, :, :n] = avk[idx].T
                conf = idx // A
                slots = np.arange(n)
                cmh[c, sg, slots % 128, (slots // 128) * CBLK + conf] = 1.0

    # bias folding (float64): h_model = h_device + (c - alpha)
    W1f, W2f = np.asarray(W1, np.float64), np.asarray(W2, np.float64)
    W3f, W4f = np.asarray(W3, np.float64), np.asarray(W4, np.float64)
    b1f, b2f = np.asarray(b1, np.float64), np.asarray(b2, np.float64)
    b3f, b4f = np.asarray(b3, np.float64), np.asarray(b4, np.float64)

    c1 = b1f
    c2 = b2f + np.einsum("sk,skj->sj", c1 - ALPHA, W2f)
    c3 = b3f + np.einsum("sk,skj->sj", c2 - ALPHA, W3f)
    c4 = b4f + np.einsum("sk,skj->sj", c3 - ALPHA, W4f)   # [S,1]

    def f32(x):
        return np.ascontiguousarray(x, dtype=np.float32)

    shared = {
        "w1": np.ascontiguousarray(W1f.astype(BF16)),
        "w2": np.ascontiguousarray(W2f.astype(BF16)),
        "w2b3": np.ascontiguousarray(
            np.tile(W2f[:, 128:, :], (1, 3, 1)).astype(BF16)),
        "w3": np.ascontiguousarray(W3f.astype(BF16)),
        "w4": np.ascontiguousarray(W4f.astype(BF16)),
        "bias_a": f32(np.concatenate([
            (INV_ALPHA * c1[:, :128] + LN_ALPHA).T,
            (-c1[:, :128]).T,
            (INV_ALPHA * c2 + LN_ALPHA).T,
            (-c2).T,
            np.broadcast_to(c4[:, 0][None, :], (128, S)),
        ], axis=1)),
        "bias_b": f32(np.concatenate([
            np.tile(INV_ALPHA * c1[:, 128:] + LN_ALPHA, (1, 3)).T,
            np.tile(-c1[:, 128:], (1, 3)).T,
            np.pad((INV_ALPHA * c3 + LN_ALPHA), ((0, 0), (0, 0))).T,
            (-c3).T,
        ], axis=1)),
    }
    in_maps = [dict(shared, xs=xs[c], cm=cmh[c]) for c in range(NCORES)]
    return PB, in_maps


def run(inputs, trace=False):
    """Build, run on 8 cores; returns ((species, energies), exec_time_ns)."""
    from concourse.bass_utils import run_bass_kernel_spmd

    PB, in_maps = _prep_inputs(**inputs)
    nc = _build(PB)
    res = run_bass_kernel_spmd(nc, in_maps, core_ids=list(range(NCORES)),
                               trace=trace)
    energies = np.concatenate(
        [np.asarray(res.results[c]["out"], np.float32).reshape(-1)
         for c in range(NCORES)])
    return (np.asarray(inputs["species"]), energies), res.exec_time_ns


def kernel(**inputs):
    out, _ = run(inputs, trace=False)
    return out


# revision 22
# speedup vs baseline: 1.1224x; 1.1224x over previous
"""ANI-style per-species MLP (MoE routing) on 8 Trainium2 NeuronCores.

Strategy:
  - Data-parallel over conformations: 256 conformations per core.
  - Routing done host-side as part of sharding: within each
    (core, 64-conformation block), atoms are stably grouped by species and
    padded to a fixed slot count PB. The device then runs dense per-species
    MLPs over contiguous slot segments (1x compute instead of 4x).
  - Per-conformation sums are computed on device with a one-hot
    slot->conformation matmul (pad slots have all-zero rows, so their
    garbage outputs never reach the energies).
  - CELU(x, a) = relu(x+c) + min(a*exp((x+c)/a), a) - a:
    one ScalarE Exp (bias/scale folded into the activation instruction),
    one VectorE tensor_scalar min, one fused VectorE scalar_tensor_tensor
    (max-with--c then add). Constant offsets (c - a) are folded into the
    next layer's bias on the host.
"""

import sys

if "/opt/trn_rl_repo" not in sys.path:
    sys.path.insert(0, "/opt/trn_rl_repo")

import numpy as np
import ml_dtypes

# ---- problem constants (hardcoded per spec) --------------------------------
S, D, H1, H2, H3 = 4, 384, 160, 128, 96
B, A = 2048, 64
ALPHA = 0.1
LN_ALPHA = float(np.log(ALPHA))
INV_ALPHA = 1.0 / ALPHA

NCORES = 8
BC = B // NCORES           # conformations per core (256)
NBLK = 4                   # conformation blocks per core
CBLK = BC // NBLK          # conformations per block (64)
APB = CBLK * A             # atoms per block (4096)
SG = NBLK * S              # segments per core (16), sg = k*S + s
TILE = 384                 # slots per compute tile
PB0 = 1152                 # default padded slots per (species, block) segment

BF16 = ml_dtypes.bfloat16

_BUILD_CACHE = {}


def _register_celu_tail():
    """Register a fused custom DVE op: out = max(in0, s0) + min(in1, s1).

    This computes the whole CELU tail (relu part + clamped exp part) in a
    single VectorE instruction; s0 is the per-partition -c bias AP.
    """
    from concourse import dve_ops
    from concourse.dve_spec import Spec, Src0, Src1, C0, C1, maxx, minn, lower
    from concourse.dve_uop import DveOpSpec

    for op in dve_ops.OPS:
        if op.name == "CELU_TAIL_ANT":
            return op
    spec = Spec(
        body=maxx(Src0, C0) + minn(Src1, C1),
        reference=lambda in0, in1, s0, s1, imm2:
            np.maximum(in0, s0) + np.minimum(in1, s1),
    )
    row = dve_ops._CUSTOM_DVE_ROW_BASE + len(dve_ops.OPS)
    shas = {}
    for ver in ("v3", "v4"):
        tmp = DveOpSpec(name="CELU_TAIL_ANT", opcode=row,
                        uops=lower(spec, ver=ver), rd1_en=True)
        shas[ver] = tmp.sha(ver)
    op = dve_ops.DveOp("CELU_TAIL_ANT", spec, subdim=False, uops_sha=shas)
    dve_ops.OPS.append(op)
    dve_ops._SUB_OPCODE_FOR_NAME[op.name] = row
    dve_ops.CUSTOM_DVE_SPECS[op.name] = spec
    return op


def _build(PB):
    """Build + compile the per-core Bass graph for padded segment size PB."""
    if PB in _BUILD_CACHE:
        return _BUILD_CACHE[PB]

    import concourse.bass as bass
    import concourse.bacc as bacc
    import concourse.mybir as mybir
    import concourse.tile as tile
    from contextlib import ExitStack

    dt = mybir.dt
    F32, BF = dt.float32, dt.bfloat16
    AF = mybir.ActivationFunctionType
    OP = mybir.AluOpType

    NT = PB // TILE            # tiles per segment
    NJ = PB // 128             # 128-slot chunks per segment
    JPT = TILE // 128          # 128-slot chunks per tile
    H1B = H1 - 128             # second-chunk width of layer-1 output (32)

    nc = bacc.Bacc("TRN2", target_bir_lowering=False, debug=False,
                   num_devices=NCORES)

    xs = nc.declare_dram_parameter("xs", [SG, D, PB], BF, isOutput=False)
    cm = nc.declare_dram_parameter("cm", [SG, 128, NJ * CBLK], BF,
                                   isOutput=False)
    w1 = nc.declare_dram_parameter("w1", [S, D, H1], BF, isOutput=False)
    w2 = nc.declare_dram_parameter("w2", [S, H1, H2], BF, isOutput=False)
    w2b3 = nc.declare_dram_parameter("w2b3", [S, 3 * (H1 - 128), H2], BF,
                                     isOutput=False)
    w3 = nc.declare_dram_parameter("w3", [S, H2, H3], BF, isOutput=False)
    w4 = nc.declare_dram_parameter("w4", [S, H3, 1], BF, isOutput=False)
    # consolidated pre-folded biases (partition-major, one column per kind/s)
    bias_a = nc.declare_dram_parameter("bias_a", [128, 5 * S], F32,
                                       isOutput=False)
    bias_b = nc.declare_dram_parameter("bias_b", [3 * H1B, 4 * S], F32,
                                       isOutput=False)
    out = nc.declare_dram_parameter("out", [NBLK, CBLK], F32, isOutput=True)

    with tile.TileContext(nc) as tc:
        with ExitStack() as ctx:
            wpool = ctx.enter_context(tc.tile_pool(name="wpool", bufs=1))
            xpool = ctx.enter_context(tc.tile_pool(name="xpool", bufs=3))
            cpool = ctx.enter_context(tc.tile_pool(name="cpool", bufs=3))
            hpool = ctx.enter_context(tc.tile_pool(name="hpool", bufs=4))
            opool = ctx.enter_context(tc.tile_pool(name="opool", bufs=2))
            pp = ctx.enter_context(
                tc.tile_pool(name="pp", bufs=1, space=bass.MemorySpace.PSUM))
            pp2 = ctx.enter_context(
                tc.tile_pool(name="pp2", bufs=2, space=bass.MemorySpace.PSUM))

            # ---- load weights + biases once (persistent tiles) ----
            _weng = [nc.sync, nc.scalar]
            _wi = [0]

            def wtile(shape, dtp, tag, src):
                t_ = wpool.tile(shape, dtp, tag=tag, name=tag)
                _weng[_wi[0] % 2].dma_start(t_[:], src)
                _wi[0] += 1
                return t_

            w1t, w2at, w2bt, w3t, w4t = {}, {}, {}, {}, {}
            b1ea, b1eb, b1nat, b1nb = {}, {}, {}, {}
            b2et, b2nt, b3et, b3nt, c4tt = {}, {}, {}, {}, {}
            for s in range(S):
                for kc in range(3):
                    w1t[s, kc] = wtile([128, H1], BF, f"w1_{s}_{kc}",
                                       w1[s, kc * 128:(kc + 1) * 128, :])
                w2at[s] = wtile([128, H2], BF, f"w2a_{s}", w2[s, 0:128, :])
                w2bt[s] = wtile([3 * H1B, H2], BF, f"w2b_{s}",
                                w2b3[s, :, :])
                w3t[s] = wtile([128, H3], BF, f"w3_{s}", w3[s, :, :])
                w4t[s] = wtile([H3, 1], BF, f"w4_{s}", w4[s, :, :])
                b1ea[s] = wtile([128, 1], F32, f"b1ea_{s}",
                                bias_a[:, 0 * S + s:0 * S + s + 1])
                b1nat[s] = wtile([128, 1], F32, f"b1na_{s}",
                                 bias_a[:, 1 * S + s:1 * S + s + 1])
                b2et[s] = wtile([128, 1], F32, f"b2e_{s}",
                                bias_a[:, 2 * S + s:2 * S + s + 1])
                b2nt[s] = wtile([128, 1], F32, f"b2n_{s}",
                                bias_a[:, 3 * S + s:3 * S + s + 1])
                c4tt[s] = wtile([128, 1], F32, f"c4_{s}",
                                bias_a[:, 4 * S + s:4 * S + s + 1])
                b1eb[s] = wtile([3 * H1B, 1], F32, f"b1eb_{s}",
                                bias_b[:, 0 * S + s:0 * S + s + 1])
                b1nb[s] = wtile([3 * H1B, 1], F32, f"b1nb_{s}",
                                bias_b[:, 1 * S + s:1 * S + s + 1])
                b3et[s] = wtile([H3, 1], F32, f"b3e_{s}",
                                bias_b[0:H3, 2 * S + s:2 * S + s + 1])
                b3nt[s] = wtile([H3, 1], F32, f"b3n_{s}",
                                bias_b[0:H3, 3 * S + s:3 * S + s + 1])

            celu_tail = _register_celu_tail()

            def celu(z, be_ap, bn_ap, mode, tagp, P, FD):
                """h' = max(z, -c) + min(a*exp((z+c)/a), a); model offset
                c - alpha is folded into the next layer's bias (bn_ap=-c)."""
                E = hpool.tile([P, FD], BF, tag=f"{tagp}_E", name=f"{tagp}_E")
                nc.scalar.activation(E[:], z[:], AF.Exp,
                                     bias=be_ap, scale=INV_ALPHA)
                h = hpool.tile([P, FD], BF, tag=f"{tagp}_h", name=f"{tagp}_h")
                nc.vector._custom_dve(celu_tail, out=h[:], in0=z[:],
                                      in1=E[:], s0=bn_ap, s1=ALPHA)
                return h

            # ---- main loop ----
            for k in range(NBLK):
                pe = pp.tile([1, CBLK], F32, tag="pe", name="pe")
                emm = 0
                for s in range(S):
                    sg = k * S + s
                    ct = cpool.tile([128, NJ * CBLK], BF, tag="ct", name="ct")
                    nc.sync.dma_start(ct[:], cm[sg, :, :])
                    # stage the whole segment's X (feature-major), one DMA
                    # per 128-feature chunk, triggered on the gpsimd queue
                    xseg = []
                    for kc in range(3):
                        xt = xpool.tile([128, PB], BF, tag=f"x{kc}",
                                        name=f"x{kc}")
                        nc.gpsimd.dma_start(
                            xt[:], xs[sg, kc * 128:(kc + 1) * 128, :])
                        xseg.append(xt)
                    # layer-1 second chunk (32 rows) merged across the
                    # segment's tiles via column tiling -> one celu
                    p1b = pp.tile([32 * NT, TILE], F32, tag="p1b",
                                  name="p1b")
                    for kc in range(3):
                        for t in range(NT):
                            nc.tensor.matmul(
                                p1b[32 * t:32 * (t + 1), :],
                                w1t[s, kc][:, 128:H1],
                                xseg[kc][:, t * TILE:(t + 1) * TILE],
                                start=(kc == 0), stop=(kc == 2),
                                tile_position=(0, 32 * t))
                    h1b = celu(p1b, b1eb[s][:], b1nb[s][:],
                               "h1b", "dve", 32 * NT, TILE)
                    for t in range(NT):
                        tsl = slice(t * TILE, (t + 1) * TILE)
                        p1a = pp2.tile([128, TILE], F32, tag="p1a",
                                       name="p1a")
                        for kc in range(3):
                            nc.tensor.matmul(p1a[:], w1t[s, kc][:, 0:128],
                                             xseg[kc][:, tsl],
                                             start=(kc == 0), stop=(kc == 2))
                        h1a = celu(p1a, b1ea[s][:], b1nat[s][:], "dve",
                                   "h1a", 128, TILE)
                        p2 = pp2.tile([128, TILE], F32, tag="p2", name="p2")
                        nc.tensor.matmul(p2[:], w2at[s][:], h1a[:],
                                         start=True, stop=False)
                        nc.tensor.matmul(p2[:],
                                         w2bt[s][32 * t:32 * (t + 1), :],
                                         h1b[32 * t:32 * (t + 1), :],
                                         start=False, stop=True)
                        h2 = celu(p2, b2et[s][:], b2nt[s][:], "dve",
                                  "h2", 128, TILE)
                        p3 = pp.tile([H3, TILE], F32, tag="p3", name="p3")
                        nc.tensor.matmul(p3[:], w3t[s][:], h2[:],
                                         start=True, stop=True)
                        h3 = celu(p3, b3et[s][:], b3nt[s][:], "dve",
                                  "h3", H3, TILE)
                        py = pp.tile([128, JPT], F32, tag="py", name="py")
                        for c in range(JPT):
                            nc.tensor.matmul(py[:, c:c + 1],
                                             h3[:, c * 128:(c + 1) * 128],
                                             w4t[s][:],
                                             start=True, stop=True)
                        ysb = hpool.tile([128, JPT], BF, tag="ysb",
                                         name="ysb")
                        nc.scalar.activation(ysb[:], py[:], AF.Identity,
                                             bias=c4tt[s])
                        for c in range(JPT):
                            j = t * JPT + c
                            nc.tensor.matmul(
                                pe[:], ysb[:, c:c + 1],
                                ct[:, j * CBLK:(j + 1) * CBLK],
                                start=(emm == 0), stop=(emm == S * NJ - 1))
                            emm += 1
                esb = opool.tile([1, CBLK], F32, tag="esb", name="esb")
                nc.vector.tensor_copy(esb[:], pe[:])
                nc.sync.dma_start(out[k:k + 1, :], esb[:])

    nc.compile()
    _BUILD_CACHE[PB] = nc
    return nc


def _prep_inputs(species, aev, W1, b1, W2, b2, W3, b3, W4, b4):
    """Host-side routing/sharding + bias folding. Returns (PB, in_maps)."""
    species = np.asarray(species)
    aev = np.asarray(aev, dtype=np.float32)

    sp = species.reshape(NCORES, NBLK, APB)
    av = aev.reshape(NCORES, NBLK, APB, D)

    # padded segment size (multiple of TILE), robust to species-count skew
    counts = np.zeros((NCORES, NBLK, S), dtype=np.int64)
    for s in range(S):
        counts[:, :, s] = (sp == s).sum(axis=2)
    maxc = int(counts.max())
    PB = max(PB0, -(-maxc // TILE) * TILE)
    NJ = PB // 128

    xs = np.zeros((NCORES, SG, D, PB), dtype=BF16)
    cmh = np.zeros((NCORES, SG, 128, NJ * CBLK), dtype=BF16)
    for c in range(NCORES):
        for k in range(NBLK):
            spk = sp[c, k]
            avk = av[c, k]
            for s in range(S):
                idx = np.flatnonzero(spk == s)
                n = idx.size
                sg = k * S + s
                xs[c, sg, :, :n] = avk[idx].T
                conf = idx // A
                slots = np.arange(n)
                cmh[c, sg, slots % 128, (slots // 128) * CBLK + conf] = 1.0

    # bias folding (float64): h_model = h_device + (c - alpha)
    W1f, W2f = np.asarray(W1, np.float64), np.asarray(W2, np.float64)
    W3f, W4f = np.asarray(W3, np.float64), np.asarray(W4, np.float64)
    b1f, b2f = np.asarray(b1, np.float64), np.asarray(b2, np.float64)
    b3f, b4f = np.asarray(b3, np.float64), np.asarray(b4, np.float64)

    c1 = b1f
    c2 = b2f + np.einsum("sk,skj->sj", c1 - ALPHA, W2f)
    c3 = b3f + np.einsum("sk,skj->sj", c2 - ALPHA, W3f)
    c4 = b4f + np.einsum("sk,skj->sj", c3 - ALPHA, W4f)   # [S,1]

    def f32(x):
        return np.ascontiguousarray(x, dtype=np.float32)

    shared = {
        "w1": np.ascontiguousarray(W1f.astype(BF16)),
        "w2": np.ascontiguousarray(W2f.astype(BF16)),
        "w2b3": np.ascontiguousarray(
            np.tile(W2f[:, 128:, :], (1, 3, 1)).astype(BF16)),
        "w3": np.ascontiguousarray(W3f.astype(BF16)),
        "w4": np.ascontiguousarray(W4f.astype(BF16)),
        "bias_a": f32(np.concatenate([
            (INV_ALPHA * c1[:, :128] + LN_ALPHA).T,
            (-c1[:, :128]).T,
            (INV_ALPHA * c2 + LN_ALPHA).T,
            (-c2).T,
            np.broadcast_to(c4[:, 0][None, :], (128, S)),
        ], axis=1)),
        "bias_b": f32(np.concatenate([
            np.tile(INV_ALPHA * c1[:, 128:] + LN_ALPHA, (1, 3)).T,
            np.tile(-c1[:, 128:], (1, 3)).T,
            np.pad((INV_ALPHA * c3 + LN_ALPHA), ((0, 0), (0, 0))).T,
            (-c3).T,
        ], axis=1)),
    }
    in_maps = [dict(shared, xs=xs[c], cm=cmh[c]) for c in range(NCORES)]
    return PB, in_maps


def run(inputs, trace=False):
    """Build, run on 8 cores; returns ((species, energies), exec_time_ns)."""
    from concourse.bass_utils import run_bass_kernel_spmd

    PB, in_maps = _prep_inputs(**inputs)
    nc = _build(PB)
    res = run_bass_kernel_spmd(nc, in_maps, core_ids=list(range(NCORES)),
                               trace=trace)
    energies = np.concatenate(
        [np.asarray(res.results[c]["out"], np.float32).reshape(-1)
         for c in range(NCORES)])
    return (np.asarray(inputs["species"]), energies), res.exec_time_ns


def kernel(**inputs):
    out, _ = run(inputs, trace=False)
    return out


# revision 27
# speedup vs baseline: 1.1614x; 1.0347x over previous
"""ANI-style per-species MLP (MoE routing) on 8 Trainium2 NeuronCores.

Strategy:
  - Data-parallel over conformations: 256 conformations per core.
  - Routing done host-side as part of sharding: within each
    (core, 64-conformation block), atoms are stably grouped by species and
    padded to a fixed slot count PB. The device then runs dense per-species
    MLPs over contiguous slot segments (1x compute instead of 4x).
  - Per-conformation sums are computed on device with a one-hot
    slot->conformation matmul (pad slots have all-zero rows, so their
    garbage outputs never reach the energies).
  - CELU(x, a) = relu(x+c) + min(a*exp((x+c)/a), a) - a:
    one ScalarE Exp (bias/scale folded into the activation instruction),
    one VectorE tensor_scalar min, one fused VectorE scalar_tensor_tensor
    (max-with--c then add). Constant offsets (c - a) are folded into the
    next layer's bias on the host.
"""

import sys

if "/opt/trn_rl_repo" not in sys.path:
    sys.path.insert(0, "/opt/trn_rl_repo")

import numpy as np
import ml_dtypes

# ---- problem constants (hardcoded per spec) --------------------------------
S, D, H1, H2, H3 = 4, 384, 160, 128, 96
B, A = 2048, 64
ALPHA = 0.1
LN_ALPHA = float(np.log(ALPHA))
INV_ALPHA = 1.0 / ALPHA

NCORES = 8
BC = B // NCORES           # conformations per core (256)
NBLK = 4                   # conformation blocks per core
CBLK = BC // NBLK          # conformations per block (64)
APB = CBLK * A             # atoms per block (4096)
SG = NBLK * S              # segments per core (16), sg = k*S + s
TILE = 384                 # slots per compute tile
PB0 = 1152                 # default padded slots per (species, block) segment

BF16 = ml_dtypes.bfloat16

_BUILD_CACHE = {}


def _register_celu_tail():
    """Register a fused custom DVE op: out = max(in0, s0) + min(in1, s1).

    This computes the whole CELU tail (relu part + clamped exp part) in a
    single VectorE instruction; s0 is the per-partition -c bias AP.
    """
    from concourse import dve_ops
    from concourse.dve_spec import Spec, Src0, Src1, C0, C1, maxx, minn, lower
    from concourse.dve_uop import DveOpSpec

    for op in dve_ops.OPS:
        if op.name == "CELU_TAIL_ANT":
            return op
    spec = Spec(
        body=maxx(Src0, C0) + minn(Src1, C1),
        reference=lambda in0, in1, s0, s1, imm2:
            np.maximum(in0, s0) + np.minimum(in1, s1),
    )
    row = dve_ops._CUSTOM_DVE_ROW_BASE + len(dve_ops.OPS)
    shas = {}
    for ver in ("v3", "v4"):
        tmp = DveOpSpec(name="CELU_TAIL_ANT", opcode=row,
                        uops=lower(spec, ver=ver), rd1_en=True)
        shas[ver] = tmp.sha(ver)
    op = dve_ops.DveOp("CELU_TAIL_ANT", spec, subdim=False, uops_sha=shas)
    dve_ops.OPS.append(op)
    dve_ops._SUB_OPCODE_FOR_NAME[op.name] = row
    dve_ops.CUSTOM_DVE_SPECS[op.name] = spec
    return op


def _build(PB):
    """Build + compile the per-core Bass graph for padded segment size PB."""
    if PB in _BUILD_CACHE:
        return _BUILD_CACHE[PB]

    import concourse.bass as bass
    import concourse.bacc as bacc
    import concourse.mybir as mybir
    import concourse.tile as tile
    from contextlib import ExitStack

    dt = mybir.dt
    F32, BF = dt.float32, dt.bfloat16
    AF = mybir.ActivationFunctionType
    OP = mybir.AluOpType

    NT = PB // TILE            # tiles per segment
    NJ = PB // 128             # 128-slot chunks per segment
    JPT = TILE // 128          # 128-slot chunks per tile
    H1B = H1 - 128             # second-chunk width of layer-1 output (32)

    nc = bacc.Bacc("TRN2", target_bir_lowering=False, debug=False,
                   num_devices=NCORES)

    xs = nc.declare_dram_parameter("xs", [SG, 128, 3 * PB], BF,
                                   isOutput=False)
    cm = nc.declare_dram_parameter("cm", [128, SG * NJ * CBLK], BF,
                                   isOutput=False)
    w1 = nc.declare_dram_parameter("w1", [S, D, H1], BF, isOutput=False)
    w2 = nc.declare_dram_parameter("w2", [S, H1, H2], BF, isOutput=False)
    w2b3 = nc.declare_dram_parameter("w2b3", [S, 3 * (H1 - 128), H2], BF,
                                     isOutput=False)
    w3 = nc.declare_dram_parameter("w3", [S, H2, H3], BF, isOutput=False)
    w4 = nc.declare_dram_parameter("w4", [S, H3, 1], BF, isOutput=False)
    # consolidated pre-folded biases (partition-major, one column per kind/s)
    bias_a = nc.declare_dram_parameter("bias_a", [128, 5 * S], F32,
                                       isOutput=False)
    bias_b = nc.declare_dram_parameter("bias_b", [3 * H1B, 4 * S], F32,
                                       isOutput=False)
    out = nc.declare_dram_parameter("out", [NBLK, CBLK], F32, isOutput=True)

    with tile.TileContext(nc) as tc:
        with ExitStack() as ctx:
            wpool = ctx.enter_context(tc.tile_pool(name="wpool", bufs=1))
            xpool = ctx.enter_context(tc.tile_pool(name="xpool", bufs=3))
            cpool = ctx.enter_context(tc.tile_pool(name="cpool", bufs=3))
            hpool = ctx.enter_context(tc.tile_pool(name="hpool", bufs=4))
            opool = ctx.enter_context(tc.tile_pool(name="opool", bufs=2))
            pp = ctx.enter_context(
                tc.tile_pool(name="pp", bufs=1, space=bass.MemorySpace.PSUM))
            pp2 = ctx.enter_context(
                tc.tile_pool(name="pp2", bufs=2, space=bass.MemorySpace.PSUM))

            # ---- load weights + biases once (persistent tiles) ----
            _weng = [nc.sync, nc.scalar]
            _wi = [0]

            def wtile(shape, dtp, tag, src):
                t_ = wpool.tile(shape, dtp, tag=tag, name=tag)
                _weng[_wi[0] % 2].dma_start(t_[:], src)
                _wi[0] += 1
                return t_

            w1t, w2at, w2bt, w3t, w4t = {}, {}, {}, {}, {}
            b1ea, b1eb, b1nat, b1nb = {}, {}, {}, {}
            b2et, b2nt, b3et, b3nt, c4tt = {}, {}, {}, {}, {}
            for s in range(S):
                for kc in range(3):
                    w1t[s, kc] = wtile([128, H1], BF, f"w1_{s}_{kc}",
                                       w1[s, kc * 128:(kc + 1) * 128, :])
                w2at[s] = wtile([128, H2], BF, f"w2a_{s}", w2[s, 0:128, :])
                w2bt[s] = wtile([3 * H1B, H2], BF, f"w2b_{s}",
                                w2b3[s, :, :])
                w3t[s] = wtile([128, H3], BF, f"w3_{s}", w3[s, :, :])
                w4t[s] = wtile([H3, 1], BF, f"w4_{s}", w4[s, :, :])
                b1ea[s] = wtile([128, 1], F32, f"b1ea_{s}",
                                bias_a[:, 0 * S + s:0 * S + s + 1])
                b1nat[s] = wtile([128, 1], F32, f"b1na_{s}",
                                 bias_a[:, 1 * S + s:1 * S + s + 1])
                b2et[s] = wtile([128, 1], F32, f"b2e_{s}",
                                bias_a[:, 2 * S + s:2 * S + s + 1])
                b2nt[s] = wtile([128, 1], F32, f"b2n_{s}",
                                bias_a[:, 3 * S + s:3 * S + s + 1])
                c4tt[s] = wtile([128, 1], F32, f"c4_{s}",
                                bias_a[:, 4 * S + s:4 * S + s + 1])
                b1eb[s] = wtile([3 * H1B, 1], F32, f"b1eb_{s}",
                                bias_b[:, 0 * S + s:0 * S + s + 1])
                b1nb[s] = wtile([3 * H1B, 1], F32, f"b1nb_{s}",
                                bias_b[:, 1 * S + s:1 * S + s + 1])
                b3et[s] = wtile([H3, 1], F32, f"b3e_{s}",
                                bias_b[0:H3, 2 * S + s:2 * S + s + 1])
                b3nt[s] = wtile([H3, 1], F32, f"b3n_{s}",
                                bias_b[0:H3, 3 * S + s:3 * S + s + 1])

            celu_tail = _register_celu_tail()

            def celu(z, be_ap, bn_ap, mode, tagp, P, FD):
                """h' = max(z, -c) + min(a*exp((z+c)/a), a); model offset
                c - alpha is folded into the next layer's bias (bn_ap=-c)."""
                E = hpool.tile([P, FD], BF, tag=f"{tagp}_E", name=f"{tagp}_E")
                nc.scalar.activation(E[:], z[:], AF.Exp,
                                     bias=be_ap, scale=INV_ALPHA)
                h = hpool.tile([P, FD], BF, tag=f"{tagp}_h", name=f"{tagp}_h")
                nc.vector._custom_dve(celu_tail, out=h[:], in0=z[:],
                                      in1=E[:], s0=bn_ap, s1=ALPHA)
                return h

            # ---- main loop ----
            for k in range(NBLK):
                pe = pp.tile([1, CBLK], F32, tag="pe", name="pe")
                emm = 0
                for s in range(S):
                    sg = k * S + s
                    ct = cpool.tile([128, NJ * CBLK], BF, tag="ct", name="ct")
                    nc.sync.dma_start(
                        ct[:], cm[:, sg * NJ * CBLK:(sg + 1) * NJ * CBLK])
                    # stage the whole segment's X (feature-major), one DMA
                    # per 128-feature chunk, triggered on the gpsimd queue
                    xt = xpool.tile([128, 3, PB], BF, tag="xseg",
                                    name="xseg")
                    nc.gpsimd.dma_start(
                        xt[:], xs[sg].rearrange("p (kc q) -> p kc q", kc=3))
                    xseg = [xt[:, kc, :] for kc in range(3)]
                    # layer-1 second chunk (32 rows) merged across the
                    # segment's tiles via column tiling -> one celu
                    p1b = pp.tile([32 * NT, TILE], F32, tag="p1b",
                                  name="p1b")
                    for kc in range(3):
                        for t in range(NT):
                            nc.tensor.matmul(
                                p1b[32 * t:32 * (t + 1), :],
                                w1t[s, kc][:, 128:H1],
                                xseg[kc][:, t * TILE:(t + 1) * TILE],
                                start=(kc == 0), stop=(kc == 2),
                                tile_position=(0, 32 * t))
                    h1b = celu(p1b, b1eb[s][:], b1nb[s][:],
                               "h1b", "dve", 32 * NT, TILE)
                    for t in range(NT):
                        tsl = slice(t * TILE, (t + 1) * TILE)
                        p1a = pp2.tile([128, TILE], F32, tag="p1a",
                                       name="p1a")
                        for kc in range(3):
                            nc.tensor.matmul(p1a[:], w1t[s, kc][:, 0:128],
                                             xseg[kc][:, tsl],
                                             start=(kc == 0), stop=(kc == 2))
                        h1a = celu(p1a, b1ea[s][:], b1nat[s][:], "dve",
                                   "h1a", 128, TILE)
                        p2 = pp2.tile([128, TILE], F32, tag="p2", name="p2")
                        nc.tensor.matmul(p2[:], w2at[s][:], h1a[:],
                                         start=True, stop=False)
                        nc.tensor.matmul(p2[:],
                                         w2bt[s][32 * t:32 * (t + 1), :],
                                         h1b[32 * t:32 * (t + 1), :],
                                         start=False, stop=True)
                        h2 = celu(p2, b2et[s][:], b2nt[s][:], "dve",
                                  "h2", 128, TILE)
                        p3 = pp.tile([H3, TILE], F32, tag="p3", name="p3")
                        nc.tensor.matmul(p3[:], w3t[s][:], h2[:],
                                         start=True, stop=True)
                        h3 = celu(p3, b3et[s][:], b3nt[s][:], "dve",
                                  "h3", H3, TILE)
                        py = pp.tile([128, JPT], F32, tag="py", name="py")
                        for c in range(JPT):
                            nc.tensor.matmul(py[:, c:c + 1],
                                             h3[:, c * 128:(c + 1) * 128],
                                             w4t[s][:],
                                             start=True, stop=True)
                        ysb = hpool.tile([128, JPT], BF, tag="ysb",
                                         name="ysb")
                        nc.scalar.activation(ysb[:], py[:], AF.Identity,
                                             bias=c4tt[s])
                        for c in range(JPT):
                            j = t * JPT + c
                            nc.tensor.matmul(
                                pe[:], ysb[:, c:c + 1],
                                ct[:, j * CBLK:(j + 1) * CBLK],
                                start=(emm == 0), stop=(emm == S * NJ - 1))
                            emm += 1
                esb = opool.tile([1, CBLK], F32, tag="esb", name="esb")
                nc.vector.tensor_copy(esb[:], pe[:])
                nc.sync.dma_start(out[k:k + 1, :], esb[:])

    nc.compile()
    _BUILD_CACHE[PB] = nc
    return nc


def _prep_inputs(species, aev, W1, b1, W2, b2, W3, b3, W4, b4):
    """Host-side routing/sharding + bias folding. Returns (PB, in_maps)."""
    species = np.asarray(species)
    aev = np.asarray(aev, dtype=np.float32)

    sp = species.reshape(NCORES, NBLK, APB)
    av = aev.reshape(NCORES, NBLK, APB, D)

    # padded segment size (multiple of TILE), robust to species-count skew
    counts = np.zeros((NCORES, NBLK, S), dtype=np.int64)
    for s in range(S):
        counts[:, :, s] = (sp == s).sum(axis=2)
    maxc = int(counts.max())
    PB = max(PB0, -(-maxc // TILE) * TILE)
    NJ = PB // 128

    xs = np.zeros((NCORES, SG, 3, 128, PB), dtype=BF16)
    cmh = np.zeros((NCORES, 128, SG, NJ * CBLK), dtype=BF16)
    xsv = xs.reshape(NCORES, SG, D, PB)  # (kc,128) = feature index
    for c in range(NCORES):
        for k in range(NBLK):
            spk = sp[c, k]
            avk = av[c, k]
            for s in range(S):
                idx = np.flatnonzero(spk == s)
                n = idx.size
                sg = k * S + s
                xsv[c, sg, :, :n] = avk[idx].T
                conf = idx // A
                slots = np.arange(n)
                cmh[c, slots % 128, sg, (slots // 128) * CBLK + conf] = 1.0
    # device expects xs[sg] as [128, 3*PB] with partition-major rows:
    # xs[c, sg, kc, p, :] -> [c, sg, p, kc, :]
    xs = np.ascontiguousarray(xs.transpose(0, 1, 3, 2, 4)).reshape(
        NCORES, SG, 128, 3 * PB)
    cmh = cmh.reshape(NCORES, 128, SG * NJ * CBLK)

    # bias folding (float64): h_model = h_device + (c - alpha)
    W1f, W2f = np.asarray(W1, np.float64), np.asarray(W2, np.float64)
    W3f, W4f = np.asarray(W3, np.float64), np.asarray(W4, np.float64)
    b1f, b2f = np.asarray(b1, np.float64), np.asarray(b2, np.float64)
    b3f, b4f = np.asarray(b3, np.float64), np.asarray(b4, np.float64)

    c1 = b1f
    c2 = b2f + np.einsum("sk,skj->sj", c1 - ALPHA, W2f)
    c3 = b3f + np.einsum("sk,skj->sj", c2 - ALPHA, W3f)
    c4 = b4f + np.einsum("sk,skj->sj", c3 - ALPHA, W4f)   # [S,1]

    def f32(x):
        return np.ascontiguousarray(x, dtype=np.float32)

    shared = {
        "w1": np.ascontiguousarray(W1f.astype(BF16)),
        "w2": np.ascontiguousarray(W2f.astype(BF16)),
        "w2b3": np.ascontiguousarray(
            np.tile(W2f[:, 128:, :], (1, 3, 1)).astype(BF16)),
        "w3": np.ascontiguousarray(W3f.astype(BF16)),
        "w4": np.ascontiguousarray(W4f.astype(BF16)),
        "bias_a": f32(np.concatenate([
            (INV_ALPHA * c1[:, :128] + LN_ALPHA).T,
            (-c1[:, :128]).T,
            (INV_ALPHA * c2 + LN_ALPHA).T,
            (-c2).T,
            np.broadcast_to(c4[:, 0][None, :], (128, S)),
        ], axis=1)),
        "bias_b": f32(np.concatenate([
            np.tile(INV_ALPHA * c1[:, 128:] + LN_ALPHA, (1, 3)).T,
            np.tile(-c1[:, 128:], (1, 3)).T,
            np.pad((INV_ALPHA * c3 + LN_ALPHA), ((0, 0), (0, 0))).T,
            (-c3).T,
        ], axis=1)),
    }
    in_maps = [dict(shared, xs=xs[c], cm=cmh[c]) for c in range(NCORES)]
    return PB, in_maps


def run(inputs, trace=False):
    """Build, run on 8 cores; returns ((species, energies), exec_time_ns)."""
    from concourse.bass_utils import run_bass_kernel_spmd

    PB, in_maps = _prep_inputs(**inputs)
    nc = _build(PB)
    res = run_bass_kernel_spmd(nc, in_maps, core_ids=list(range(NCORES)),
                               trace=trace)
    energies = np.concatenate(
        [np.asarray(res.results[c]["out"], np.float32).reshape(-1)
         for c in range(NCORES)])
    return (np.asarray(inputs["species"]), energies), res.exec_time_ns


def kernel(**inputs):
    out, _ = run(inputs, trace=False)
    return out


# revision 34
# speedup vs baseline: 1.3168x; 1.1338x over previous
"""ANI-style per-species MLP (MoE routing) on 8 Trainium2 NeuronCores.

Strategy:
  - Data-parallel over conformations: 256 conformations per core.
  - Routing done host-side as part of sharding: within each
    (core, 64-conformation block), atoms are stably grouped by species and
    padded to a fixed slot count PB. The device then runs dense per-species
    MLPs over contiguous slot segments (1x compute instead of 4x).
  - Per-conformation sums are computed on device with a one-hot
    slot->conformation matmul (pad slots have all-zero rows, so their
    garbage outputs never reach the energies).
  - CELU(x, a) = relu(x+c) + min(a*exp((x+c)/a), a) - a:
    one ScalarE Exp (bias/scale folded into the activation instruction),
    one VectorE tensor_scalar min, one fused VectorE scalar_tensor_tensor
    (max-with--c then add). Constant offsets (c - a) are folded into the
    next layer's bias on the host.
"""

import sys

if "/opt/trn_rl_repo" not in sys.path:
    sys.path.insert(0, "/opt/trn_rl_repo")

import numpy as np
import ml_dtypes

# ---- problem constants (hardcoded per spec) --------------------------------
S, D, H1, H2, H3 = 4, 384, 160, 128, 96
B, A = 2048, 64
ALPHA = 0.1
LN_ALPHA = float(np.log(ALPHA))
INV_ALPHA = 1.0 / ALPHA

NCORES = 8
BC = B // NCORES           # conformations per core (256)
NBLK = 4                   # conformation blocks per core
CBLK = BC // NBLK          # conformations per block (64)
APB = CBLK * A             # atoms per block (4096)
SG = NBLK * S              # segments per core (16), sg = k*S + s
TILE = 384                 # slots per compute tile
PB0 = 1152                 # default padded slots per (species, block) segment

H1B = H1 - 128             # second-chunk width of layer-1 output (32)
BF16 = ml_dtypes.bfloat16

_BUILD_CACHE = {}

# packed weight blob column offsets (bf16 words), shared host/device
def _wblob_map():
    off = 0
    m = {}
    for s in range(S):
        for kc in range(3):
            m[("w1", s, kc)] = off; off += H1
        m[("w2a", s)] = off; off += H2
        m[("w2b", s)] = off; off += H2
        m[("w3", s)] = off; off += H3
        m[("w4", s)] = off; off += 1
    return m, off

WMAP, WCOLS = _wblob_map()


def _register_celu_tail():
    """Register a fused custom DVE op: out = max(in0, s0) + min(in1, s1).

    This computes the whole CELU tail (relu part + clamped exp part) in a
    single VectorE instruction; s0 is the per-partition -c bias AP.
    """
    from concourse import dve_ops
    from concourse.dve_spec import Spec, Src0, Src1, C0, C1, maxx, minn, lower
    from concourse.dve_uop import DveOpSpec

    for op in dve_ops.OPS:
        if op.name == "CELU_TAIL_ANT":
            return op
    spec = Spec(
        body=maxx(Src0, C0) + minn(Src1, C1),
        reference=lambda in0, in1, s0, s1, imm2:
            np.maximum(in0, s0) + np.minimum(in1, s1),
    )
    row = dve_ops._CUSTOM_DVE_ROW_BASE + len(dve_ops.OPS)
    shas = {}
    for ver in ("v3", "v4"):
        tmp = DveOpSpec(name="CELU_TAIL_ANT", opcode=row,
                        uops=lower(spec, ver=ver), rd1_en=True)
        shas[ver] = tmp.sha(ver)
    op = dve_ops.DveOp("CELU_TAIL_ANT", spec, subdim=False, uops_sha=shas)
    dve_ops.OPS.append(op)
    dve_ops._SUB_OPCODE_FOR_NAME[op.name] = row
    dve_ops.CUSTOM_DVE_SPECS[op.name] = spec
    return op


def _build(PB):
    """Build + compile the per-core Bass graph for padded segment size PB."""
    if PB in _BUILD_CACHE:
        return _BUILD_CACHE[PB]

    import concourse.bass as bass
    import concourse.bacc as bacc
    import concourse.mybir as mybir
    import concourse.tile as tile
    from contextlib import ExitStack

    dt = mybir.dt
    F32, BF = dt.float32, dt.bfloat16
    AF = mybir.ActivationFunctionType
    OP = mybir.AluOpType

    NT = PB // TILE            # tiles per segment
    NJ = PB // 128             # 128-slot chunks per segment
    JPT = TILE // 128          # 128-slot chunks per tile
    H1B = H1 - 128             # second-chunk width of layer-1 output (32)

    nc = bacc.Bacc("TRN2", target_bir_lowering=False, debug=False,
                   num_devices=NCORES)

    xs = nc.declare_dram_parameter("xs", [SG, 128, 3 * PB], BF,
                                   isOutput=False)
    cm = nc.declare_dram_parameter("cm", [128, SG * NJ * CBLK], BF,
                                   isOutput=False)
    wbl = nc.declare_dram_parameter("wbl", [128, WCOLS], BF, isOutput=False)
    # consolidated pre-folded biases (partition-major, one column per kind/s)
    bias_a = nc.declare_dram_parameter("bias_a", [128, 5 * S], F32,
                                       isOutput=False)
    bias_b = nc.declare_dram_parameter("bias_b", [3 * H1B, 4 * S], F32,
                                       isOutput=False)
    out = nc.declare_dram_parameter("out", [NBLK, CBLK], F32, isOutput=True)
    dump = nc.dram_tensor("dump", [1, 8], F32)

    with tile.TileContext(nc) as tc:
        with ExitStack() as ctx:
            wpool = ctx.enter_context(tc.tile_pool(name="wpool", bufs=1))
            xpool = ctx.enter_context(tc.tile_pool(name="xpool", bufs=4))
            cpool = ctx.enter_context(tc.tile_pool(name="cpool", bufs=3))
            hpool = ctx.enter_context(tc.tile_pool(name="hpool", bufs=5))
            opool = ctx.enter_context(tc.tile_pool(name="opool", bufs=2))
            pp = ctx.enter_context(
                tc.tile_pool(name="pp", bufs=1, space=bass.MemorySpace.PSUM))
            pp2 = ctx.enter_context(
                tc.tile_pool(name="pp2", bufs=2, space=bass.MemorySpace.PSUM))

            # ---- load weights: ONE packed DMA + two bias DMAs ----
            wblob = wpool.tile([128, WCOLS], BF, tag="wblob", name="wblob")
            nc.sync.dma_start(wblob[:], wbl[:, :])
            bias_a_t = wpool.tile([128, 5 * S], F32, tag="bias_a",
                                  name="bias_a")
            nc.sync.dma_start(bias_a_t[:], bias_a[:, :])
            bias_b_t = wpool.tile([3 * H1B, 4 * S], F32, tag="bias_b",
                                  name="bias_b")
            nc.sync.dma_start(bias_b_t[:], bias_b[:, :])

            # PE warm-up: ~5us of junk matmuls during the DMA-bound
            # startup so the HAM clock gate reaches 8/8 before real work
            pwarm = pp2.tile([128, 512], F32, tag="p1a", name="pwarm")
            for wv in range(24):
                nc.tensor.matmul(pwarm[:], wblob[:, 0:128],
                                 wblob[:, 0:512], start=True, stop=True)
            wsb = opool.tile([1, 8], F32, tag="wsb", name="wsb")
            nc.vector.tensor_copy(wsb[:], pwarm[0:1, 0:8])
            nc.sync.dma_start(dump[:, :], wsb[:])

            def wsl(key, lo, hi, plo=0, phi=128):
                o = WMAP[key]
                return wblob[plo:phi, o + lo:o + hi]

            w1t = {(s, kc): wsl(("w1", s, kc), 0, H1)
                   for s in range(S) for kc in range(3)}
            w2at = {s: wsl(("w2a", s), 0, H2) for s in range(S)}
            w2bt = {s: wsl(("w2b", s), 0, H2, 0, 3 * H1B) for s in range(S)}
            w3t = {s: wsl(("w3", s), 0, H3) for s in range(S)}
            w4t = {s: wsl(("w4", s), 0, 1, 0, H3) for s in range(S)}
            b1ea = {s: bias_a_t[:, s:s + 1] for s in range(S)}
            b1nat = {s: bias_a_t[:, S + s:S + s + 1] for s in range(S)}
            b2et = {s: bias_a_t[:, 2 * S + s:2 * S + s + 1] for s in range(S)}
            b2nt = {s: bias_a_t[:, 3 * S + s:3 * S + s + 1] for s in range(S)}
            c4tt = {s: bias_a_t[:, 4 * S + s:4 * S + s + 1] for s in range(S)}
            b1eb = {s: bias_b_t[:, s:s + 1] for s in range(S)}
            b1nb = {s: bias_b_t[:, S + s:S + s + 1] for s in range(S)}
            b3et = {s: bias_b_t[0:H3, 2 * S + s:2 * S + s + 1]
                    for s in range(S)}
            b3nt = {s: bias_b_t[0:H3, 3 * S + s:3 * S + s + 1]
                    for s in range(S)}

            celu_tail = _register_celu_tail()

            def celu(z, be_ap, bn_ap, mode, tagp, P, FD):
                """h' = max(z, -c) + min(a*exp((z+c)/a), a); model offset
                c - alpha is folded into the next layer's bias (bn_ap=-c)."""
                E = hpool.tile([P, FD], BF, tag=f"{tagp}_E", name=f"{tagp}_E")
                nc.scalar.activation(E[:], z[:], AF.Exp,
                                     bias=be_ap, scale=INV_ALPHA)
                h = hpool.tile([P, FD], BF, tag=f"{tagp}_h", name=f"{tagp}_h")
                nc.vector._custom_dve(celu_tail, out=h[:], in0=z[:],
                                      in1=E[:], s0=bn_ap, s1=ALPHA)
                return h

            # ---- main loop ----
            for k in range(NBLK):
                pe = pp.tile([1, CBLK], F32, tag="pe", name="pe")
                emm = 0
                for s in range(S):
                    sg = k * S + s
                    ct = cpool.tile([128, NJ * CBLK], BF, tag="ct", name="ct")
                    nc.sync.dma_start(
                        ct[:], cm[:, sg * NJ * CBLK:(sg + 1) * NJ * CBLK])
                    # stage the whole segment's X (feature-major), one DMA
                    # per 128-feature chunk, triggered on the gpsimd queue
                    xt = xpool.tile([128, 3, PB], BF, tag="xseg",
                                    name="xseg")
                    nc.gpsimd.dma_start(
                        xt[:], xs[sg].rearrange("p (kc q) -> p kc q", kc=3))
                    xseg = [xt[:, kc, :] for kc in range(3)]
                    # layer-1 second chunk (32 rows) merged across the
                    # segment's tiles via column tiling -> one celu
                    p1b = pp.tile([32 * NT, TILE], F32, tag="p1b",
                                  name="p1b")
                    for kc in range(3):
                        for t in range(NT):
                            nc.tensor.matmul(
                                p1b[32 * t:32 * (t + 1), :],
                                w1t[s, kc][:, 128:H1],
                                xseg[kc][:, t * TILE:(t + 1) * TILE],
                                start=(kc == 0), stop=(kc == 2),
                                tile_position=(0, 32 * t))
                    h1b = celu(p1b, b1eb[s], b1nb[s],
                               "h1b", "dve", 32 * NT, TILE)
                    for t in range(NT):
                        tsl = slice(t * TILE, (t + 1) * TILE)
                        p1a = pp2.tile([128, TILE], F32, tag="p1a",
                                       name="p1a")
                        for kc in range(3):
                            nc.tensor.matmul(p1a[:], w1t[s, kc][:, 0:128],
                                             xseg[kc][:, tsl],
                                             start=(kc == 0), stop=(kc == 2))
                        h1a = celu(p1a, b1ea[s], b1nat[s], "dve",
                                   "h1a", 128, TILE)
                        p2 = pp2.tile([128, TILE], F32, tag="p2", name="p2")
                        nc.tensor.matmul(p2[:], w2at[s], h1a[:],
                                         start=True, stop=False)
                        nc.tensor.matmul(p2[:],
                                         w2bt[s][32 * t:32 * (t + 1), :],
                                         h1b[32 * t:32 * (t + 1), :],
                                         start=False, stop=True)
                        h2 = celu(p2, b2et[s], b2nt[s], "dve",
                                  "h2", 128, TILE)
                        p3 = pp.tile([H3, TILE], F32, tag="p3", name="p3")
                        nc.tensor.matmul(p3[:], w3t[s], h2[:],
                                         start=True, stop=True)
                        h3 = celu(p3, b3et[s], b3nt[s], "dve",
                                  "h3", H3, TILE)
                        py = pp.tile([128, JPT], F32, tag="py", name="py")
                        for c in range(JPT):
                            nc.tensor.matmul(py[:, c:c + 1],
                                             h3[:, c * 128:(c + 1) * 128],
                                             w4t[s],
                                             start=True, stop=True)
                        ysb = hpool.tile([128, JPT], BF, tag="ysb",
                                         name="ysb")
                        nc.scalar.activation(ysb[:], py[:], AF.Identity,
                                             bias=c4tt[s])
                        for c in range(JPT):
                            j = t * JPT + c
                            nc.tensor.matmul(
                                pe[:], ysb[:, c:c + 1],
                                ct[:, j * CBLK:(j + 1) * CBLK],
                                start=(emm == 0), stop=(emm == S * NJ - 1))
                            emm += 1
                esb = opool.tile([1, CBLK], F32, tag="esb", name="esb")
                nc.vector.tensor_copy(esb[:], pe[:])
                nc.sync.dma_start(out[k:k + 1, :], esb[:])

    nc.compile()
    _BUILD_CACHE[PB] = nc
    return nc


def _prep_inputs(species, aev, W1, b1, W2, b2, W3, b3, W4, b4):
    """Host-side routing/sharding + bias folding. Returns (PB, in_maps)."""
    species = np.asarray(species)
    aev = np.asarray(aev, dtype=np.float32)

    sp = species.reshape(NCORES, NBLK, APB)
    av = aev.reshape(NCORES, NBLK, APB, D)

    # padded segment size (multiple of TILE), robust to species-count skew
    counts = np.zeros((NCORES, NBLK, S), dtype=np.int64)
    for s in range(S):
        counts[:, :, s] = (sp == s).sum(axis=2)
    maxc = int(counts.max())
    PB = max(PB0, -(-maxc // TILE) * TILE)
    NJ = PB // 128

    xs = np.zeros((NCORES, SG, 3, 128, PB), dtype=BF16)
    cmh = np.zeros((NCORES, 128, SG, NJ * CBLK), dtype=BF16)
    xsv = xs.reshape(NCORES, SG, D, PB)  # (kc,128) = feature index
    for c in range(NCORES):
        for k in range(NBLK):
            spk = sp[c, k]
            avk = av[c, k]
            for s in range(S):
                idx = np.flatnonzero(spk == s)
                n = idx.size
                sg = k * S + s
                xsv[c, sg, :, :n] = avk[idx].T
                conf = idx // A
                slots = np.arange(n)
                cmh[c, slots % 128, sg, (slots // 128) * CBLK + conf] = 1.0
    # device expects xs[sg] as [128, 3*PB] with partition-major rows:
    # xs[c, sg, kc, p, :] -> [c, sg, p, kc, :]
    xs = np.ascontiguousarray(xs.transpose(0, 1, 3, 2, 4)).reshape(
        NCORES, SG, 128, 3 * PB)
    cmh = cmh.reshape(NCORES, 128, SG * NJ * CBLK)

    # bias folding (float64): h_model = h_device + (c - alpha)
    W1f, W2f = np.asarray(W1, np.float64), np.asarray(W2, np.float64)
    W3f, W4f = np.asarray(W3, np.float64), np.asarray(W4, np.float64)
    b1f, b2f = np.asarray(b1, np.float64), np.asarray(b2, np.float64)
    b3f, b4f = np.asarray(b3, np.float64), np.asarray(b4, np.float64)

    c1 = b1f
    c2 = b2f + np.einsum("sk,skj->sj", c1 - ALPHA, W2f)
    c3 = b3f + np.einsum("sk,skj->sj", c2 - ALPHA, W3f)
    c4 = b4f + np.einsum("sk,skj->sj", c3 - ALPHA, W4f)   # [S,1]

    def f32(x):
        return np.ascontiguousarray(x, dtype=np.float32)

    wbl = np.zeros((128, WCOLS), dtype=BF16)
    for s in range(S):
        for kc in range(3):
            o = WMAP[("w1", s, kc)]
            wbl[:, o:o + H1] = W1f[s, kc * 128:(kc + 1) * 128, :]
        o = WMAP[("w2a", s)]
        wbl[:, o:o + H2] = W2f[s, 0:128, :]
        o = WMAP[("w2b", s)]
        wbl[0:3 * H1B, o:o + H2] = np.tile(W2f[s, 128:, :], (3, 1))
        o = WMAP[("w3", s)]
        wbl[:, o:o + H3] = W3f[s]
        o = WMAP[("w4", s)]
        wbl[0:H3, o:o + 1] = W4f[s]
    shared = {
        "wbl": wbl,
        "bias_a": f32(np.concatenate([
            (INV_ALPHA * c1[:, :128] + LN_ALPHA).T,
            (-c1[:, :128]).T,
            (INV_ALPHA * c2 + LN_ALPHA).T,
            (-c2).T,
            np.broadcast_to(c4[:, 0][None, :], (128, S)),
        ], axis=1)),
        "bias_b": f32(np.concatenate([
            np.tile(INV_ALPHA * c1[:, 128:] + LN_ALPHA, (1, 3)).T,
            np.tile(-c1[:, 128:], (1, 3)).T,
            np.pad((INV_ALPHA * c3 + LN_ALPHA), ((0, 0), (0, 0))).T,
            (-c3).T,
        ], axis=1)),
    }
    in_maps = [dict(shared, xs=xs[c], cm=cmh[c]) for c in range(NCORES)]
    return PB, in_maps


def run(inputs, trace=False):
    """Build, run on 8 cores; returns ((species, energies), exec_time_ns)."""
    from concourse.bass_utils import run_bass_kernel_spmd

    PB, in_maps = _prep_inputs(**inputs)
    nc = _build(PB)
    last_err = None
    for attempt in range(4):
        try:
            res = run_bass_kernel_spmd(nc, in_maps,
                                       core_ids=list(range(NCORES)),
                                       trace=trace)
            energies = np.concatenate(
                [np.asarray(res.results[c]["out"], np.float32).reshape(-1)
                 for c in range(NCORES)])
            return ((np.asarray(inputs["species"]), energies),
                    res.exec_time_ns)
        except Exception as e:  # transient device hiccups: retry
            last_err = e
            import time as _time
            _time.sleep(3.0)
    raise last_err


def kernel(**inputs):
    out, _ = run(inputs, trace=False)
    return out
